# revision 78
# baseline (speedup 1.0000x reference)
"""Causal attention, key-block-parity sharding + fp8 DoubleRow everywhere the
error budget allows.

Sharding: 8 cores = 4 batches x 2 key-parities. Per batch, the 16 key blocks
(128 keys each) split into sets A={0,3,4,7,8,11,12,15}, B={1,2,5,6,9,10,13,14}
(chosen so both cores carry exactly 68 causal key-blocks). Each core computes
ALL 16 query tiles of its batch against its own key blocks (flash-style
partial softmax: unnormalized o and lsum returned; host combines
(oA+oB)/(lA+lB)). Per tile g the program processes a uniform cnt(g)=g//2+1
packed key blocks; when the core's set has fewer valid blocks (<=1 short), the
extra block is masked off via data, keeping the instruction stream identical
across cores (SPMD).

Precision/speed plan (cost-model driven; fp8 = e4m3, DoubleRow = 4x rate):
- All weights scaled x64 on host so fp8 quantization stays in normal range
  (raw |W| <= 1/32 is e4m3-subnormal). q descaled at its cast; k keeps x64
  (folded into the exp scale); v descaled at the fp16 V cast.
- Q^T proj: single-set fp8 DR. K^T proj: 3-term split-fp8 DR
  (wkh.xk full + wkl.xk and wkh.xkl at half contraction).
- V proj: 3-term split-fp8 DR (full), output fp16.
- S^T computed pre-transposed per 128-key block: qh.kh (full) + ql.kh
  and qh.kl (half contraction), fp8 DR; causal/pad mask added via an extra
  fp8 matmul (-240 I)^T @ (240 pattern) inside the accumulation group.
- exp on Act reads S^T psum, emits Pe^T fp16 directly usable as PV lhsT
  (no PE transposes, no psum round-trip); lsum via 1-col ones matmuls.
- P@V in fp16, acc fp32 psum, o out fp32.
Measured end-to-end error vs the fp32 reference (exact inputs): 1.52e-2;
hardware matches the numpy model to ~4e-5 (gate: 2e-2).
"""
from contextlib import ExitStack

import numpy as np
import ml_dtypes

import concourse.bacc as bacc
import concourse.tile as tile
import concourse.mybir as mybir

F32 = mybir.dt.float32
F16 = mybir.dt.float16
F8 = mybir.dt.float8e4

B, T, D = 4, 2048, 1024
P = 128
ND = D // P          # 8 contraction blocks
NT = T // P          # 16 query tiles
NKB = ND             # 8 packed key blocks per core (1024 keys)
# Weights are scaled x64 on host so fp8 quantization stays in normal range
# (raw |W| <= 1/32 is subnormal territory for e4m3). k keeps the x64 scale in
# SBUF; q is descaled at the fp8 cast. exp scale folds the rest: 1/(32*64).
WS = 64.0
SCALE = 1.0 / 32.0
SEXP = SCALE / WS
MASKV = -240.0 * 240.0   # additive pre-scale mask: -57600 -> -28.1 after SEXP

SETS = [[0, 3, 4, 7, 8, 11, 12, 15], [1, 2, 5, 6, 9, 10, 13, 14]]
CNT = [g // 2 + 1 for g in range(NT)]  # uniform packed-block count per tile


def _mask_kind(p, g):
    """Mask for tile g's LAST packed block on parity p:
    'pad' (block id > g, fully masked), 'diag' (== g, causal tri), 'none'."""
    s = SETS[p][CNT[g] - 1]
    return "pad" if s > g else ("diag" if s == g else "none")


def _pieces(nb):
    """Split nb blocks into psum pieces of <=4 blocks."""
    out = []
    off = 0
    while nb > 0:
        w = min(4, nb)
        out.append((off, w))
        off += w
        nb -= w
    return out


def build_program():
    nc = bacc.Bacc("TRN2", target_bir_lowering=False, debug=False)

    xt8 = nc.dram_tensor("xt8", [D, T], F8, kind="ExternalInput").ap()
    x8k = nc.dram_tensor("x8k", [D, NKB * P], F8, kind="ExternalInput").ap()
    x8kl = nc.dram_tensor("x8kl", [D, NKB * P], F8, kind="ExternalInput").ap()
    wq8 = nc.dram_tensor("wq8", [D, D], F8, kind="ExternalInput").ap()
    wkh8 = nc.dram_tensor("wkh8", [D, D], F8, kind="ExternalInput").ap()
    wkl8 = nc.dram_tensor("wkl8", [D, D], F8, kind="ExternalInput").ap()
    wvh8 = nc.dram_tensor("wvh8", [D, D], F8, kind="ExternalInput").ap()
    wvl8 = nc.dram_tensor("wvl8", [D, D], F8, kind="ExternalInput").ap()
    mb8 = nc.dram_tensor("mb8", [P, NT * P], F8, kind="ExternalInput").ap()
    ineg8 = nc.dram_tensor("ineg8", [P, P], F8, kind="ExternalInput").ap()
    out_o = nc.dram_tensor("out_o", [T, D], F32, kind="ExternalOutput").ap()
    out_l = nc.dram_tensor("out_l", [P, NT], F32, kind="ExternalOutput").ap()

    AF = mybir.ActivationFunctionType
    OP = mybir.AluOpType
    DR = mybir.MatmulPerfMode.DoubleRow

    with tile.TileContext(nc) as tc:
        with (
            tc.tile_pool(name="persist", bufs=1) as persist,
            tc.tile_pool(name="res", bufs=1, side="right") as res,
        ):
            ones16 = persist.tile([P, 1], F16)
            nc.gpsimd.memset(ones16[:], 1.0)
            ineg_sb = persist.tile([P, P], F8)
            mb_sb = persist.tile([P, NT * P], F8)
            lsb = persist.tile([P, NT], F32)

            # PE pstate warmup: dummy matmuls on a zeroed tile so the 3us
            # clock ramp completes while the first DMAs are still landing.
            dum8 = persist.tile([P, 512], F8)
            nc.gpsimd.memset(dum8[:], 0.0)
            with tc.tile_pool(name="warm", bufs=1, space="PSUM") as warmp:
                wps = warmp.tile([P, 512], F32)
                for w in range(10):
                    nc.tensor.matmul(
                        wps[:], dum8[:, 0:P], dum8[:],
                        start=(w == 0), stop=(w == 9),
                    )

            # persistent projection results
            qh = res.tile([P, ND, T], F8)
            ql = res.tile([P, ND, T], F8)
            kh = res.tile([P, ND, NKB * P], F8)
            kl = res.tile([P, ND, NKB * P], F8)
            V16 = res.tile([P, NKB, D], F16)

            es_in = ExitStack()
            xp = es_in.enter_context(tc.tile_pool(name="xp", bufs=1))
            wp = es_in.enter_context(tc.tile_pool(name="wp", bufs=1))
            x8_sb = xp.tile([P, ND, T], F8)
            xk_sb = xp.tile([P, ND, NKB * P], F8)
            xkl_sb = xp.tile([P, ND, NKB * P], F8)
            wq_sb = wp.tile([P, ND, D], F8)
            wkh_sb = wp.tile([P, ND, D], F8)
            wkl_sb = wp.tile([P, 4, D], F8)
            wvh_sb = wp.tile([P, ND, D], F8)
            wvl_sb = wp.tile([P, ND, D], F8)
            # DMA issue order tracks first consumption by the proj weave:
            # Q(s0) pieces, [K,Q,Q] rounds, then [K,V,Q,V] rounds. One DMA
            # per tensor(-half) with a strided 3D AP (dst [P, ND, cols]).
            def dma(dst, src, c0, c1):
                nc.sync.dma_start(
                    dst[:, :, c0:c1],
                    src.rearrange("(i p) c -> p i c", p=P)[:, :, c0:c1],
                )

            # preload the activation table while the first DMAs land
            nc.scalar.activation(
                out=ones16[:, 0:1], in_=ones16[:, 0:1], func=AF.Copy
            )
            dma(wq_sb, wq8, 0, 512)
            dma(x8_sb, xt8, 0, 512)
            dma(wq_sb, wq8, 512, D)
            dma(x8_sb, xt8, 512, 1024)
            dma(wkh_sb, wkh8, 0, D)
            dma(xk_sb, x8k, 0, 512)
            nc.sync.dma_start(
                wkl_sb[:],
                wkl8.rearrange("(i p) c -> p i c", p=P)[:, 0:4, :],
            )
            dma(xkl_sb, x8kl, 0, 512)
            dma(x8_sb, xt8, 1024, 1536)
            dma(wvh_sb, wvh8, 0, 512)
            dma(wvl_sb, wvl8, 0, 512)
            dma(xk_sb, x8k, 512, 1024)
            dma(xkl_sb, x8kl, 512, 1024)
            dma(wvh_sb, wvh8, 512, D)
            dma(wvl_sb, wvl8, 512, D)
            dma(x8_sb, xt8, 1536, 2048)
            nc.sync.dma_start(ineg_sb[:], ineg8[:])
            nc.sync.dma_start(mb_sb[:], mb8[:])

            with (
                tc.tile_pool(name="pep", bufs=4) as pep,
                tc.tile_pool(name="osb", bufs=2) as osb,
            ):
                es_pp = ExitStack()
                pp = es_pp.enter_context(
                    tc.tile_pool(name="pp", bufs=8, space="PSUM")
                )

                k_state = {}

                def emit_K_hi(kp, oo):
                    """K^T proj hi-term: (wkh . xk), full contraction."""
                    cols = slice(512 * kp, 512 * (kp + 1))
                    ocols = slice(P * oo, P * (oo + 1))
                    pk = pp.tile([P, 512], F32, tag="pp")
                    for ii in range(ND // 2):
                        nc.tensor.matmul(
                            pk[:], wkh_sb[:, 2 * ii:2 * ii + 2, ocols],
                            xk_sb[:, 2 * ii:2 * ii + 2, cols],
                            start=(ii == 0), stop=False, perf_mode=DR,
                        )
                    k_state[(kp, oo)] = pk

                def emit_K_lo(kp, oo):
                    """K^T proj lo-corrections at half contraction + casts."""
                    cols = slice(512 * kp, 512 * (kp + 1))
                    ocols = slice(P * oo, P * (oo + 1))
                    pk = k_state.pop((kp, oo))
                    for ii in (0, 1):
                        nc.tensor.matmul(
                            pk[:], wkl_sb[:, 2 * ii:2 * ii + 2, ocols],
                            xk_sb[:, 2 * ii:2 * ii + 2, cols],
                            start=False, stop=False, perf_mode=DR,
                        )
                    for n, ii in enumerate((2, 3)):
                        nc.tensor.matmul(
                            pk[:], wkh_sb[:, 2 * ii:2 * ii + 2, ocols],
                            xkl_sb[:, 2 * ii:2 * ii + 2, cols],
                            start=False, stop=(n == 1), perf_mode=DR,
                        )
                    nc.scalar.activation(out=kh[:, oo, cols], in_=pk[:], func=AF.Copy)
                    if oo >= 4:
                        # kl only feeds the half-contraction S correction
                        nc.vector.tensor_tensor(
                            out=kl[:, oo, cols], in0=pk[:], in1=kh[:, oo, cols],
                            op=OP.subtract,
                        )

                def emit_K(kp, oo):
                    emit_K_hi(kp, oo)
                    emit_K_lo(kp, oo)

                def emit_Q(s, oo):
                    """Q^T proj piece (fp8 DoubleRow) -> split-fp8 qh + ql."""
                    cols = slice(512 * s, 512 * (s + 1))
                    pq = pp.tile([P, 512], F32, tag="pp")
                    for ii in range(ND // 2):
                        nc.tensor.matmul(
                            pq[:], wq_sb[:, 2 * ii:2 * ii + 2, P * oo:P * (oo + 1)],
                            x8_sb[:, 2 * ii:2 * ii + 2, cols],
                            start=(ii == 0), stop=(ii == ND // 2 - 1),
                            perf_mode=DR,
                        )
                    # psum holds 64*q; q stored unscaled in fp8
                    nc.scalar.activation(
                        out=qh[:, oo, cols], in_=pq[:], func=AF.Copy,
                        scale=1.0 / WS,
                    )
                    if oo < 4:
                        # ql only feeds the half-contraction S correction
                        nc.vector.scalar_tensor_tensor(
                            out=ql[:, oo, cols], in0=pq[:], scalar=1.0 / WS,
                            in1=qh[:, oo, cols], op0=OP.mult, op1=OP.subtract,
                        )

                def emit_V(vb, h):
                    """V proj piece: 3-term split-fp8 DoubleRow -> fp16 V."""
                    kcols = slice(P * vb, P * (vb + 1))
                    hc = slice(512 * h, 512 * (h + 1))
                    pv = pp.tile([P, 512], F32, tag="pp")
                    n = 0
                    for x_, w_ in ((xk_sb, wvh_sb), (xkl_sb, wvh_sb),
                                   (xk_sb, wvl_sb)):
                        for ii in range(ND // 2):
                            n += 1
                            nc.tensor.matmul(
                                pv[:], x_[:, 2 * ii:2 * ii + 2, kcols],
                                w_[:, 2 * ii:2 * ii + 2, hc],
                                start=(n == 1), stop=(n == 12), perf_mode=DR,
                            )
                    nc.scalar.activation(
                        out=V16[:, vb, hc], in_=pv[:], func=AF.Copy, scale=1.0 / WS
                    )

                # Interleave pieces so per-round cast work stays under PE work
                # and DMA arrival order is respected: Q slabs 0-1 first (small
                # dep set), K hi-parts as soon as wkh+xk land, then rounds.
                units = [("Q", (s, oo)) for s in range(2) for oo in range(ND)]
                units += [("Kh", (0, oo)) for oo in range(6)]
                qq = [("Q", (s, oo)) for s in range(2, 4) for oo in range(ND)]
                vv = ([("V", (vb, 0)) for vb in range(NKB)]
                      + [("V", (vb, 1)) for vb in range(NKB)])
                for r in range(6):
                    units += [("Kl", (0, r)), qq[r], vv[r]]
                for r in range(6, 8):
                    units += [("K", (0, r)), qq[r], vv[r]]
                for r in range(8):
                    units += [("K", (1, r)), qq[8 + r], vv[8 + r]]
                emitters = {"K": emit_K, "Kh": emit_K_hi, "Kl": emit_K_lo,
                            "Q": emit_Q, "V": emit_V}
                for kind, args in units:
                    emitters[kind](*args)

                es_pp.close()

                # ---- Attention: software pipeline over tiles (descending) ----
                es_at = ExitStack()
                ps = es_at.enter_context(
                    tc.tile_pool(name="ps", bufs=3, space="PSUM")
                )
                acclp = es_at.enter_context(
                    tc.tile_pool(name="acclp", bufs=1, space="PSUM")
                )
                accp = es_at.enter_context(
                    tc.tile_pool(name="accp", bufs=2, space="PSUM")
                )
                state = {}

                def emit_S(g):
                    nb = CNT[g]
                    qcols = slice(P * g, P * (g + 1))
                    piece_tiles = []
                    for (off, pw) in _pieces(nb):
                        # S^T pieces: psum [P(keys), pw, 128(q)]; per block 12
                        # DR matmuls with k-side as lhsT -> scores land
                        # pre-transposed, no PE transpose / pt copy needed.
                        pst = ps.tile([P, 4, P], F32, tag="ps")
                        qcols = slice(P * g, P * (g + 1))
                        for b in range(pw):
                            kcols = slice(P * (off + b), P * (off + b + 1))
                            is_diag = off + b == nb - 1
                            n = 0
                            # lo-corrections run at half contraction depth
                            for kt_, qt_, iis in (
                                (kh, qh, (0, 1, 2, 3)), (kh, ql, (0, 1)),
                                (kl, qh, (2, 3)),
                            ):
                                for ii in iis:
                                    n += 1
                                    if is_diag and n == 8:
                                        # additive causal/pad mask, emitted
                                        # inside the group so a DR matmul
                                        # carries the stop flag
                                        nc.tensor.matmul(
                                            pst[:, b, :], ineg_sb[:],
                                            mb_sb[:, P * g:P * (g + 1)],
                                            start=False, stop=False,
                                            skip_group_check=True,
                                        )
                                    nc.tensor.matmul(
                                        pst[:, b, :],
                                        kt_[:, 2 * ii:2 * ii + 2, kcols],
                                        qt_[:, 2 * ii:2 * ii + 2, qcols],
                                        start=(n == 1),
                                        stop=(n == 8),
                                        perf_mode=DR,
                                    )
                        piece_tiles.append((pst, off, pw))
                    state[g] = piece_tiles

                def emit_tail(g):
                    nb = CNT[g]
                    piece_tiles = state.pop(g)
                    pet = []
                    for (pst, off, pw) in piece_tiles:
                        pe = pep.tile([P, 4, P], F16, tag="pe")
                        nc.scalar.activation(
                            out=pe[:, 0:pw, :], in_=pst[:, 0:pw, :],
                            func=AF.Exp, bias=0.0, scale=SEXP,
                        )
                        for b in range(pw):
                            pet.append(pe[:, b, :])
                    # lsum[q] = sum_k PeT[k, q] via 1-col ones matmuls
                    accl = acclp.tile([P, 1], F32, tag="accl")
                    for blk in range(nb):
                        nc.tensor.matmul(
                            accl[:], pet[blk], ones16[:],
                            start=(blk == 0), stop=(blk == nb - 1),
                        )
                    nc.vector.tensor_copy(lsb[:, g:g + 1], accl[:])
                    acc = accp.tile([P, D], F32, tag="acc")
                    o_sb = osb.tile([P, D], F32, tag="o")
                    for h in range(2):
                        hc = slice(512 * h, 512 * (h + 1))
                        for blk in range(nb):
                            nc.tensor.matmul(
                                acc[:, hc],
                                pet[blk], V16[:, blk, hc],
                                start=(blk == 0), stop=(blk == nb - 1),
                            )
                        nc.vector.tensor_copy(o_sb[:, hc], acc[:, hc])
                        nc.sync.dma_start(
                            out_o[P * g:P * (g + 1), hc], o_sb[:, hc]
                        )

                # Strictly alternate small (1 psum piece) and big (2 piece)
                # tiles so at most 3 S-piece psums are in flight; start with
                # tile 7 (its q slab is ready before the last proj rounds).
                order = []
                for i in range(NT // 2):
                    order += [7 - i, NT - 1 - i]
                emit_S(order[0])
                for idx in range(1, NT):
                    emit_S(order[idx])
                    emit_tail(order[idx - 1])
                emit_tail(order[-1])
                nc.sync.dma_start(out_l[:], lsb[:])
                es_at.close()

            es_in.close()

    nc.compile()
    return nc


def _packed_cols(p):
    return np.concatenate([np.arange(P * s, P * (s + 1)) for s in SETS[p]])


def _mask_b8(p):
    """Per-tile [P, 128] additive-mask pattern (values 0 or 240), S^T
    orientation: rows = key within final block, cols = query within tile."""
    m = np.zeros((P, NT * P), dtype=np.float32)
    r = np.arange(P)[:, None]   # key index
    f = np.arange(P)[None, :]   # query index
    for g in range(NT):
        kind = _mask_kind(p, g)
        if kind == "pad":
            m[:, P * g:P * (g + 1)] = 240.0
        elif kind == "diag":
            m[:, P * g:P * (g + 1)] = np.where(r > f, 240.0, 0.0)
    return m.astype(ml_dtypes.float8_e4m3)


def _split8(a):
    """fp8 two-term split: a ~= hi + lo with hi = fp8(a), lo = fp8(a - hi)."""
    F8 = ml_dtypes.float8_e4m3
    hi = a.astype(F8)
    lo = (a - hi.astype(np.float32)).astype(F8)
    return hi, lo


def make_in_maps(x, Wq, Wk, Wv):
    F8 = ml_dtypes.float8_e4m3
    x = np.asarray(x, dtype=np.float32)
    wqt = np.ascontiguousarray(np.asarray(Wq, np.float32).T)
    wkt = np.ascontiguousarray(np.asarray(Wk, np.float32).T)
    wvt = np.ascontiguousarray(np.asarray(Wv, np.float32).T)
    wq8 = (WS * wqt).astype(F8)
    wkh8, wkl8 = _split8(WS * wkt)
    wvh8, wvl8 = _split8(WS * wvt)
    ineg = (-240.0 * np.eye(P, dtype=np.float32)).astype(F8)
    masks = [_mask_b8(0), _mask_b8(1)]
    cols = [_packed_cols(0), _packed_cols(1)]

    in_maps = []
    for b in range(B):
        xtb = np.ascontiguousarray(x[b].T)
        xt8 = xtb.astype(F8)
        xt8l = (xtb - xt8.astype(np.float32)).astype(F8)
        for par in range(2):
            x8k = np.ascontiguousarray(xt8[:, cols[par]])
            x8kl = np.ascontiguousarray(xt8l[:, cols[par]])
            in_maps.append(
                {"xt8": xt8, "x8k": x8k, "x8kl": x8kl, "wq8": wq8,
                 "wkh8": wkh8, "wkl8": wkl8, "wvh8": wvh8, "wvl8": wvl8,
                 "mb8": masks[par], "ineg8": ineg}
            )
    return in_maps


def assemble(results):
    out = np.empty((B, T, D), dtype=np.float32)
    for b in range(B):
        oA = results[2 * b]["out_o"]
        oB = results[2 * b + 1]["out_o"]
        lA = results[2 * b]["out_l"]
        lB = results[2 * b + 1]["out_l"]
        l = (lA + lB).T.reshape(T, 1)  # [P, NT] -> rows 128g+r
        out[b] = (oA + oB) / l
    return out


_CACHED = {}


def _get_program():
    if "nc" not in _CACHED:
        _CACHED["nc"] = build_program()
    return _CACHED["nc"]


def kernel(x, Wq, Wk, Wv):
    from concourse.bass_utils import run_bass_kernel_spmd
    res = run_bass_kernel_spmd(_get_program(), make_in_maps(x, Wq, Wk, Wv),
                               core_ids=list(range(8)))
    return assemble(res.results)


if __name__ == "__main__":
    from concourse.timeline_sim import TimelineSim
    nc = build_program()
    print("kernel7 sim:", TimelineSim(nc).simulate())


# revision 80
# speedup vs baseline: 1.0473x; 1.0473x over previous
"""Causal attention, key-block-parity sharding + fp8 DoubleRow everywhere the
error budget allows.

Sharding: 8 cores = 4 batches x 2 key-parities. Per batch, the 16 key blocks
(128 keys each) split into sets A={0,3,4,7,8,11,12,15}, B={1,2,5,6,9,10,13,14}
(chosen so both cores carry exactly 68 causal key-blocks). Each core computes
ALL 16 query tiles of its batch against its own key blocks (flash-style
partial softmax: unnormalized o and lsum returned; host combines
(oA+oB)/(lA+lB)). Per tile g the program processes a uniform cnt(g)=g//2+1
packed key blocks; when the core's set has fewer valid blocks (<=1 short), the
extra block is masked off via data, keeping the instruction stream identical
across cores (SPMD).

Precision/speed plan (cost-model driven; fp8 = e4m3, DoubleRow = 4x rate):
- All weights scaled x64 on host so fp8 quantization stays in normal range
  (raw |W| <= 1/32 is e4m3-subnormal). q descaled at its cast; k keeps x64
  (folded into the exp scale); v descaled at the fp16 V cast.
- Q^T proj: single-set fp8 DR. K^T proj: 3-term split-fp8 DR
  (wkh.xk full + wkl.xk and wkh.xkl at half contraction).
- V proj: 3-term split-fp8 DR (full), output fp16.
- S^T computed pre-transposed per 128-key block: qh.kh (full) + ql.kh
  and qh.kl (half contraction), fp8 DR; causal/pad mask added via an extra
  fp8 matmul (-240 I)^T @ (240 pattern) inside the accumulation group.
- exp on Act reads S^T psum, emits Pe^T fp16 directly usable as PV lhsT
  (no PE transposes, no psum round-trip); lsum via 1-col ones matmuls.
- P@V in fp16, acc fp32 psum, o out fp32.
Measured end-to-end error vs the fp32 reference (exact inputs): 1.52e-2;
hardware matches the numpy model to ~4e-5 (gate: 2e-2).
"""
from contextlib import ExitStack

import numpy as np
import ml_dtypes

import concourse.bacc as bacc
import concourse.tile as tile
import concourse.mybir as mybir

F32 = mybir.dt.float32
F16 = mybir.dt.float16
F8 = mybir.dt.float8e4

B, T, D = 4, 2048, 1024
P = 128
ND = D // P          # 8 contraction blocks
NT = T // P          # 16 query tiles
NKB = ND             # 8 packed key blocks per core (1024 keys)
# Weights are scaled x64 on host so fp8 quantization stays in normal range
# (raw |W| <= 1/32 is subnormal territory for e4m3). k keeps the x64 scale in
# SBUF; q is descaled at the fp8 cast. exp scale folds the rest: 1/(32*64).
WS = 64.0
SCALE = 1.0 / 32.0
SEXP = SCALE / WS
MASKV = -240.0 * 240.0   # additive pre-scale mask: -57600 -> -28.1 after SEXP

SETS = [[0, 3, 4, 7, 8, 11, 12, 15], [1, 2, 5, 6, 9, 10, 13, 14]]
CNT = [g // 2 + 1 for g in range(NT)]  # uniform packed-block count per tile


def _mask_kind(p, g):
    """Mask for tile g's LAST packed block on parity p:
    'pad' (block id > g, fully masked), 'diag' (== g, causal tri), 'none'."""
    s = SETS[p][CNT[g] - 1]
    return "pad" if s > g else ("diag" if s == g else "none")


def _pieces(nb):
    """Split nb blocks into psum pieces of <=4 blocks."""
    out = []
    off = 0
    while nb > 0:
        w = min(4, nb)
        out.append((off, w))
        off += w
        nb -= w
    return out


def build_program():
    nc = bacc.Bacc("TRN2", target_bir_lowering=False, debug=False)

    xt8 = nc.dram_tensor("xt8", [D, T], F8, kind="ExternalInput").ap()
    x8k = nc.dram_tensor("x8k", [D, NKB * P], F8, kind="ExternalInput").ap()
    x8kl = nc.dram_tensor("x8kl", [D, NKB * P], F8, kind="ExternalInput").ap()
    wq8 = nc.dram_tensor("wq8", [D, D], F8, kind="ExternalInput").ap()
    wkh8 = nc.dram_tensor("wkh8", [D, D], F8, kind="ExternalInput").ap()
    wkl8 = nc.dram_tensor("wkl8", [D, D], F8, kind="ExternalInput").ap()
    wvh8 = nc.dram_tensor("wvh8", [D, D], F8, kind="ExternalInput").ap()
    wvl8 = nc.dram_tensor("wvl8", [D, D], F8, kind="ExternalInput").ap()
    mb8 = nc.dram_tensor("mb8", [P, NT * P], F8, kind="ExternalInput").ap()
    ineg8 = nc.dram_tensor("ineg8", [P, P], F8, kind="ExternalInput").ap()
    out_o = nc.dram_tensor("out_o", [T, D], F32, kind="ExternalOutput").ap()
    out_l = nc.dram_tensor("out_l", [P, NT], F32, kind="ExternalOutput").ap()

    AF = mybir.ActivationFunctionType
    OP = mybir.AluOpType
    DR = mybir.MatmulPerfMode.DoubleRow

    with tile.TileContext(nc) as tc:
        with (
            tc.tile_pool(name="persist", bufs=1) as persist,
            tc.tile_pool(name="res", bufs=1, side="right") as res,
        ):
            ones16 = persist.tile([P, 1], F16)
            nc.gpsimd.memset(ones16[:], 1.0)
            ineg_sb = persist.tile([P, P], F8)
            mb_sb = persist.tile([P, NT * P], F8)
            lsb = persist.tile([P, NT], F32)

            # PE pstate warmup: dummy matmuls on a zeroed tile so the 3us
            # clock ramp completes while the first DMAs are still landing.
            dum8 = persist.tile([P, 512], F8)
            nc.gpsimd.memset(dum8[:], 0.0)
            with tc.tile_pool(name="warm", bufs=1, space="PSUM") as warmp:
                wps = warmp.tile([P, 512], F32)
                for w in range(10):
                    nc.tensor.matmul(
                        wps[:], dum8[:, 0:P], dum8[:],
                        start=(w == 0), stop=(w == 9),
                    )

            # persistent projection results
            qh = res.tile([P, ND, T], F8)
            ql = res.tile([P, ND, T], F8)
            kh = res.tile([P, ND, NKB * P], F8)
            kl = res.tile([P, ND, NKB * P], F8)
            V16 = res.tile([P, NKB, D], F16)

            es_in = ExitStack()
            xp = es_in.enter_context(tc.tile_pool(name="xp", bufs=1))
            wp = es_in.enter_context(tc.tile_pool(name="wp", bufs=1))
            x8_sb = xp.tile([P, ND, T], F8)
            xk_sb = xp.tile([P, ND, NKB * P], F8)
            xkl_sb = xp.tile([P, ND, NKB * P], F8)
            wq_sb = wp.tile([P, ND, D], F8)
            wkh_sb = wp.tile([P, ND, D], F8)
            wkl_sb = wp.tile([P, 4, D], F8)
            wvh_sb = wp.tile([P, ND, D], F8)
            wvl_sb = wp.tile([P, ND, D], F8)
            # DMA issue order tracks first consumption by the proj weave:
            # Q(s0) pieces, [K,Q,Q] rounds, then [K,V,Q,V] rounds. One DMA
            # per tensor(-half) with a strided 3D AP (dst [P, ND, cols]).
            def dma(dst, src, c0, c1):
                nc.sync.dma_start(
                    dst[:, :, c0:c1],
                    src.rearrange("(i p) c -> p i c", p=P)[:, :, c0:c1],
                )

            # preload the activation table while the first DMAs land
            nc.scalar.activation(
                out=ones16[:, 0:1], in_=ones16[:, 0:1], func=AF.Copy
            )
            dma(wq_sb, wq8, 0, 512)
            dma(x8_sb, xt8, 0, 512)
            dma(wq_sb, wq8, 512, D)
            dma(x8_sb, xt8, 512, 1024)
            dma(wkh_sb, wkh8, 0, D)
            dma(xk_sb, x8k, 0, 512)
            nc.sync.dma_start(
                wkl_sb[:],
                wkl8.rearrange("(i p) c -> p i c", p=P)[:, 0:4, :],
            )
            dma(xkl_sb, x8kl, 0, 512)
            dma(x8_sb, xt8, 1024, 1536)
            dma(wvh_sb, wvh8, 0, 512)
            dma(wvl_sb, wvl8, 0, 512)
            dma(xk_sb, x8k, 512, 1024)
            dma(xkl_sb, x8kl, 512, 1024)
            dma(wvh_sb, wvh8, 512, D)
            dma(wvl_sb, wvl8, 512, D)
            dma(x8_sb, xt8, 1536, 2048)
            nc.sync.dma_start(ineg_sb[:], ineg8[:])
            nc.sync.dma_start(mb_sb[:], mb8[:])

            with (
                tc.tile_pool(name="pep", bufs=6) as pep,
                tc.tile_pool(name="osb", bufs=5) as osb,
            ):
                es_pp = ExitStack()
                pp = es_pp.enter_context(
                    tc.tile_pool(name="pp", bufs=8, space="PSUM")
                )

                k_state = {}

                def emit_K_hi(kp, oo):
                    """K^T proj hi-term: (wkh . xk), full contraction."""
                    cols = slice(512 * kp, 512 * (kp + 1))
                    ocols = slice(P * oo, P * (oo + 1))
                    pk = pp.tile([P, 512], F32, tag="pp")
                    for ii in range(ND // 2):
                        nc.tensor.matmul(
                            pk[:], wkh_sb[:, 2 * ii:2 * ii + 2, ocols],
                            xk_sb[:, 2 * ii:2 * ii + 2, cols],
                            start=(ii == 0), stop=False, perf_mode=DR,
                        )
                    k_state[(kp, oo)] = pk

                def emit_K_lo(kp, oo):
                    """K^T proj lo-corrections at half contraction + casts."""
                    cols = slice(512 * kp, 512 * (kp + 1))
                    ocols = slice(P * oo, P * (oo + 1))
                    pk = k_state.pop((kp, oo))
                    for ii in (0, 1):
                        nc.tensor.matmul(
                            pk[:], wkl_sb[:, 2 * ii:2 * ii + 2, ocols],
                            xk_sb[:, 2 * ii:2 * ii + 2, cols],
                            start=False, stop=False, perf_mode=DR,
                        )
                    for n, ii in enumerate((2, 3)):
                        nc.tensor.matmul(
                            pk[:], wkh_sb[:, 2 * ii:2 * ii + 2, ocols],
                            xkl_sb[:, 2 * ii:2 * ii + 2, cols],
                            start=False, stop=(n == 1), perf_mode=DR,
                        )
                    nc.scalar.activation(out=kh[:, oo, cols], in_=pk[:], func=AF.Copy)
                    if oo >= 4:
                        # kl only feeds the half-contraction S correction
                        nc.vector.tensor_tensor(
                            out=kl[:, oo, cols], in0=pk[:], in1=kh[:, oo, cols],
                            op=OP.subtract,
                        )

                def emit_K(kp, oo):
                    emit_K_hi(kp, oo)
                    emit_K_lo(kp, oo)

                def emit_Q(s, oo):
                    """Q^T proj piece (fp8 DoubleRow) -> split-fp8 qh + ql."""
                    cols = slice(512 * s, 512 * (s + 1))
                    pq = pp.tile([P, 512], F32, tag="pp")
                    for ii in range(ND // 2):
                        nc.tensor.matmul(
                            pq[:], wq_sb[:, 2 * ii:2 * ii + 2, P * oo:P * (oo + 1)],
                            x8_sb[:, 2 * ii:2 * ii + 2, cols],
                            start=(ii == 0), stop=(ii == ND // 2 - 1),
                            perf_mode=DR,
                        )
                    # psum holds 64*q; q stored unscaled in fp8
                    nc.scalar.activation(
                        out=qh[:, oo, cols], in_=pq[:], func=AF.Copy,
                        scale=1.0 / WS,
                    )
                    if oo < 4:
                        # ql only feeds the half-contraction S correction
                        nc.vector.scalar_tensor_tensor(
                            out=ql[:, oo, cols], in0=pq[:], scalar=1.0 / WS,
                            in1=qh[:, oo, cols], op0=OP.mult, op1=OP.subtract,
                        )

                def emit_V(vb, h):
                    """V proj piece: 3-term split-fp8 DoubleRow -> fp16 V."""
                    kcols = slice(P * vb, P * (vb + 1))
                    hc = slice(512 * h, 512 * (h + 1))
                    pv = pp.tile([P, 512], F32, tag="pp")
                    n = 0
                    for x_, w_ in ((xk_sb, wvh_sb), (xkl_sb, wvh_sb),
                                   (xk_sb, wvl_sb)):
                        for ii in range(ND // 2):
                            n += 1
                            nc.tensor.matmul(
                                pv[:], x_[:, 2 * ii:2 * ii + 2, kcols],
                                w_[:, 2 * ii:2 * ii + 2, hc],
                                start=(n == 1), stop=(n == 12), perf_mode=DR,
                            )
                    nc.scalar.activation(
                        out=V16[:, vb, hc], in_=pv[:], func=AF.Copy, scale=1.0 / WS
                    )

                # Interleave pieces so per-round cast work stays under PE work
                # and DMA arrival order is respected: Q slabs 0-1 first (small
                # dep set), K hi-parts as soon as wkh+xk land, then rounds.
                units = [("Q", (s, oo)) for s in range(2) for oo in range(ND)]
                units += [("Kh", (0, oo)) for oo in range(6)]
                qq = [("Q", (s, oo)) for s in range(2, 4) for oo in range(ND)]
                vv = ([("V", (vb, 0)) for vb in range(NKB)]
                      + [("V", (vb, 1)) for vb in range(NKB)])
                for r in range(6):
                    units += [("Kl", (0, r)), qq[r], vv[r]]
                for r in range(6, 8):
                    units += [("K", (0, r)), qq[r], vv[r]]
                for r in range(8):
                    units += [("K", (1, r)), qq[8 + r], vv[8 + r]]
                emitters = {"K": emit_K, "Kh": emit_K_hi, "Kl": emit_K_lo,
                            "Q": emit_Q, "V": emit_V}
                for kind, args in units:
                    emitters[kind](*args)

                es_pp.close()

                # ---- Attention: software pipeline over tiles (descending) ----
                es_at = ExitStack()
                ps = es_at.enter_context(
                    tc.tile_pool(name="ps", bufs=3, space="PSUM")
                )
                acclp = es_at.enter_context(
                    tc.tile_pool(name="acclp", bufs=1, space="PSUM")
                )
                accp = es_at.enter_context(
                    tc.tile_pool(name="accp", bufs=2, space="PSUM")
                )
                state = {}

                def emit_S(g):
                    nb = CNT[g]
                    qcols = slice(P * g, P * (g + 1))
                    piece_tiles = []
                    for (off, pw) in _pieces(nb):
                        # S^T pieces: psum [P(keys), pw, 128(q)]; per block 12
                        # DR matmuls with k-side as lhsT -> scores land
                        # pre-transposed, no PE transpose / pt copy needed.
                        pst = ps.tile([P, 4, P], F32, tag="ps")
                        qcols = slice(P * g, P * (g + 1))
                        for b in range(pw):
                            kcols = slice(P * (off + b), P * (off + b + 1))
                            is_diag = off + b == nb - 1
                            n = 0
                            # lo-corrections run at half contraction depth
                            for kt_, qt_, iis in (
                                (kh, qh, (0, 1, 2, 3)), (kh, ql, (0, 1)),
                                (kl, qh, (2, 3)),
                            ):
                                for ii in iis:
                                    n += 1
                                    if is_diag and n == 8:
                                        # additive causal/pad mask, emitted
                                        # inside the group so a DR matmul
                                        # carries the stop flag
                                        nc.tensor.matmul(
                                            pst[:, b, :], ineg_sb[:],
                                            mb_sb[:, P * g:P * (g + 1)],
                                            start=False, stop=False,
                                            skip_group_check=True,
                                        )
                                    nc.tensor.matmul(
                                        pst[:, b, :],
                                        kt_[:, 2 * ii:2 * ii + 2, kcols],
                                        qt_[:, 2 * ii:2 * ii + 2, qcols],
                                        start=(n == 1),
                                        stop=(n == 8),
                                        perf_mode=DR,
                                    )
                        piece_tiles.append((pst, off, pw))
                    state[g] = piece_tiles

                def emit_tail(g):
                    nb = CNT[g]
                    piece_tiles = state.pop(g)
                    pet = []
                    for (pst, off, pw) in piece_tiles:
                        pe = pep.tile([P, 4, P], F16, tag="pe")
                        nc.scalar.activation(
                            out=pe[:, 0:pw, :], in_=pst[:, 0:pw, :],
                            func=AF.Exp, bias=0.0, scale=SEXP,
                        )
                        for b in range(pw):
                            pet.append(pe[:, b, :])
                    # lsum[q] = sum_k PeT[k, q] via 1-col ones matmuls
                    accl = acclp.tile([P, 1], F32, tag="accl")
                    for blk in range(nb):
                        nc.tensor.matmul(
                            accl[:], pet[blk], ones16[:],
                            start=(blk == 0), stop=(blk == nb - 1),
                        )
                    nc.vector.tensor_copy(lsb[:, g:g + 1], accl[:])
                    acc = accp.tile([P, D], F32, tag="acc")
                    o_sb = osb.tile([P, D], F32, tag="o")
                    for h in range(2):
                        hc = slice(512 * h, 512 * (h + 1))
                        for blk in range(nb):
                            nc.tensor.matmul(
                                acc[:, hc],
                                pet[blk], V16[:, blk, hc],
                                start=(blk == 0), stop=(blk == nb - 1),
                            )
                        nc.vector.tensor_copy(o_sb[:, hc], acc[:, hc])
                        nc.sync.dma_start(
                            out_o[P * g:P * (g + 1), hc], o_sb[:, hc]
                        )

                # Strictly alternate small (1 psum piece) and big (2 piece)
                # tiles so at most 3 S-piece psums are in flight; start with
                # tile 7 (its q slab is ready before the last proj rounds).
                order = []
                for i in range(NT // 2):
                    order += [7 - i, NT - 1 - i]
                emit_S(order[0])
                for idx in range(1, NT):
                    emit_S(order[idx])
                    emit_tail(order[idx - 1])
                emit_tail(order[-1])
                nc.sync.dma_start(out_l[:], lsb[:])
                es_at.close()

            es_in.close()

    nc.compile()
    return nc


def _packed_cols(p):
    return np.concatenate([np.arange(P * s, P * (s + 1)) for s in SETS[p]])


def _mask_b8(p):
    """Per-tile [P, 128] additive-mask pattern (values 0 or 240), S^T
    orientation: rows = key within final block, cols = query within tile."""
    m = np.zeros((P, NT * P), dtype=np.float32)
    r = np.arange(P)[:, None]   # key index
    f = np.arange(P)[None, :]   # query index
    for g in range(NT):
        kind = _mask_kind(p, g)
        if kind == "pad":
            m[:, P * g:P * (g + 1)] = 240.0
        elif kind == "diag":
            m[:, P * g:P * (g + 1)] = np.where(r > f, 240.0, 0.0)
    return m.astype(ml_dtypes.float8_e4m3)


def _split8(a):
    """fp8 two-term split: a ~= hi + lo with hi = fp8(a), lo = fp8(a - hi)."""
    F8 = ml_dtypes.float8_e4m3
    hi = a.astype(F8)
    lo = (a - hi.astype(np.float32)).astype(F8)
    return hi, lo


def make_in_maps(x, Wq, Wk, Wv):
    F8 = ml_dtypes.float8_e4m3
    x = np.asarray(x, dtype=np.float32)
    wqt = np.ascontiguousarray(np.asarray(Wq, np.float32).T)
    wkt = np.ascontiguousarray(np.asarray(Wk, np.float32).T)
    wvt = np.ascontiguousarray(np.asarray(Wv, np.float32).T)
    wq8 = (WS * wqt).astype(F8)
    wkh8, wkl8 = _split8(WS * wkt)
    wvh8, wvl8 = _split8(WS * wvt)
    ineg = (-240.0 * np.eye(P, dtype=np.float32)).astype(F8)
    masks = [_mask_b8(0), _mask_b8(1)]
    cols = [_packed_cols(0), _packed_cols(1)]

    in_maps = []
    for b in range(B):
        xtb = np.ascontiguousarray(x[b].T)
        xt8 = xtb.astype(F8)
        xt8l = (xtb - xt8.astype(np.float32)).astype(F8)
        for par in range(2):
            x8k = np.ascontiguousarray(xt8[:, cols[par]])
            x8kl = np.ascontiguousarray(xt8l[:, cols[par]])
            in_maps.append(
                {"xt8": xt8, "x8k": x8k, "x8kl": x8kl, "wq8": wq8,
                 "wkh8": wkh8, "wkl8": wkl8, "wvh8": wvh8, "wvl8": wvl8,
                 "mb8": masks[par], "ineg8": ineg}
            )
    return in_maps


def assemble(results):
    out = np.empty((B, T, D), dtype=np.float32)
    for b in range(B):
        oA = results[2 * b]["out_o"]
        oB = results[2 * b + 1]["out_o"]
        lA = results[2 * b]["out_l"]
        lB = results[2 * b + 1]["out_l"]
        l = (lA + lB).T.reshape(T, 1)  # [P, NT] -> rows 128g+r
        out[b] = (oA + oB) / l
    return out


_CACHED = {}


def _get_program():
    if "nc" not in _CACHED:
        _CACHED["nc"] = build_program()
    return _CACHED["nc"]


def kernel(x, Wq, Wk, Wv):
    from concourse.bass_utils import run_bass_kernel_spmd
    res = run_bass_kernel_spmd(_get_program(), make_in_maps(x, Wq, Wk, Wv),
                               core_ids=list(range(8)))
    return assemble(res.results)


if __name__ == "__main__":
    from concourse.timeline_sim import TimelineSim
    nc = build_program()
    print("kernel7 sim:", TimelineSim(nc).simulate())
